# revision 6
# baseline (speedup 1.0000x reference)
"""GAT (4-layer, 8-head) message-passing kernel for 8 Trainium2 NeuronCores.

Strategy (self-contained; shapes hardcoded from the problem spec):
  - Nodes are sharded by destination across 8 cores (6250 nodes each);
    within each core nodes are permuted by (degree, lo-hi balance) so that
    128-node tiles have uniform padded in-degree D.
  - Per layer: each core computes hW = f @ W for its local nodes (PE),
    per-node attention coefficients asrc/adst (DVE), packs [bf16 h | f32
    asrc] into 256-byte table rows, and AllGathers the table to all cores.
  - Per-edge work uses dma_gather (256B rows) with int16 indices. The
    int16 range limit is handled with two gather bases (row 0 and row
    3*SHARD) that together cover the whole table; edges from ranks 3-4 are
    reachable from either base, which lets per-node slot assignment meet
    uniform per-tile column cuts with only a narrow double-gathered strip.
  - Slots land [node-on-partition, column]; softmax (no max-subtraction
    needed, logits are O(10)) via ACT Exp; the segment sum (numerator and
    denominator in one shot) is identity-lhsT PE matmuls accumulating
    columns into PSUM.
  - global_mean_pool: per-core onehot matmul accumulation + AllReduce,
    then the final 32->2 linear on every core (identical outputs).
"""
import sys
sys.path.insert(0, "/opt/trn_rl_repo")

import numpy as np
import ml_dtypes

BF16 = ml_dtypes.bfloat16

N = 50000
E = 1600000
G_GRAPHS = 128
F_IN = 128
HEADS, HID = 8, 8
C_HID = 64
OUT_C = 32
NC = 8
NLOC = N // NC            # 6250
SHARD = NLOC + 2          # 6252 rows per rank in the table (+Z +NEG)
NTILES = 49
NPAD = NTILES * 128
BASE1 = 3 * SHARD         # second gather base
NEG_VAL = -1e30
Z_OFF, NEG_OFF = NLOC, NLOC + 1


# ----------------------------------------------------------------------
# CPU preprocessing
# ----------------------------------------------------------------------

def _preprocess(edge_index, batch):
    # self-loops are handled analytically in the dense phase (p_self and
    # p_self*h computed on-chip), so they are excluded from the edge stream
    src = np.asarray(edge_index[0]).astype(np.int64)
    dst = np.asarray(edge_index[1]).astype(np.int64)
    batch = np.asarray(batch).astype(np.int64)
    deg = np.bincount(dst, minlength=N)

    src_rank = src // NLOC
    cls = np.where(src_rank < 3, 0, np.where(src_rank < 5, 1, 2))
    nlow_of = np.bincount(dst, weights=(cls == 0), minlength=N).astype(np.int64)
    nhigh_of = np.bincount(dst, weights=(cls == 2), minlength=N).astype(np.int64)

    order_pos = np.empty(N, np.int64)
    perm = []
    for c in range(NC):
        local = np.arange(c * NLOC, (c + 1) * NLOC)
        key = deg[local] * 4000 + (nlow_of[local] - nhigh_of[local]) + 2000
        p = local[np.argsort(-key, kind="stable")]
        perm.append(p)
        order_pos[p] = np.arange(NLOC)
    row_of = np.empty(N, np.int64)
    for c in range(NC):
        row_of[perm[c]] = c * SHARD + order_pos[perm[c]]

    core_e = dst // NLOC
    pos_e = order_pos[dst]
    srow = row_of[src]

    ndeg = np.zeros((NC, NPAD), np.int32)
    nlow = np.zeros((NC, NPAD), np.int32)
    nmid = np.zeros((NC, NPAD), np.int32)
    np.add.at(ndeg, (core_e, pos_e), 1)
    np.add.at(nlow, (core_e, pos_e), (cls == 0).astype(np.int32))
    np.add.at(nmid, (core_e, pos_e), (cls == 1).astype(np.int32))
    nhigh = ndeg - nlow - nmid

    D = np.zeros(NTILES, np.int32)
    K = np.zeros(NTILES, np.int32)
    Wd = np.zeros(NTILES, np.int32)
    for t in range(NTILES):
        s = t * 128
        real = min(128, NLOC - s) if s < NLOC else 0
        dm = int(ndeg[:, s:s + real].max()) if real else 1
        d = max(4, dm)
        Lmax = int(nlow[:, s:s + real].max()) if real else 0
        Hmax = int(nhigh[:, s:s + real].max()) if real else 0
        w = max(0, Lmax - (d - Hmax))
        k = d - Hmax
        D[t], K[t], Wd[t] = d, k, w

    key2 = ((core_e * NPAD + pos_e) * 4 + cls)
    o = np.argsort(key2, kind="stable")
    srow_s, core_s, pos_s, cls_s = srow[o], core_e[o], pos_e[o], cls[o]
    gid = core_s * NPAD + pos_s
    first = np.r_[True, gid[1:] != gid[:-1]]
    cum = np.arange(len(gid)) - np.maximum.accumulate(
        np.where(first, np.arange(len(gid)), 0))
    t_of = pos_s // 128
    col = np.where(cls_s < 2, cum,
                   D[t_of] - nhigh[core_s, pos_s] + cum
                   - nlow[core_s, pos_s] - nmid[core_s, pos_s])
    Dmax = int(D.max())
    which = np.full((NC, NPAD, Dmax), 4, np.int8)   # 4 = pad
    rowg = np.zeros((NC, NPAD, Dmax), np.int64)
    which[:, NLOC:, 0] = 3                          # dummy-node Z slot
    which[core_s, pos_s, col] = cls_s
    rowg[core_s, pos_s, col] = srow_s

    batchrel = np.full((NC, NPAD), 255, np.int64)
    for c in range(NC):
        batchrel[c, :NLOC] = batch[perm[c]]
    cnt = np.bincount(batch, minlength=G_GRAPHS).astype(np.float32)
    return dict(perm=perm, which=which, rowg=rowg, D=D, K=K, Wd=Wd,
                batchrel=batchrel, cnt=cnt)


def _build_idx_core(pp, c):
    """Linear int16 idx stream for core c: per tile, calls X, A, B, Y."""
    chunks = []
    for t in range(NTILES):
        d, k, w = int(pp["D"][t]), int(pp["K"][t]), int(pp["Wd"][t])
        wh = pp["which"][c, t * 128:(t + 1) * 128, :d]
        rg = pp["rowg"][c, t * 128:(t + 1) * 128, :d]
        rel0 = rg
        rel1 = rg - BASE1
        wx = wh[:, :k]
        iX = np.where(wx <= 1, rel0[:, :k], np.where(wx == 3, Z_OFF, NEG_OFF))
        wa = wh[:, k:k + w]
        iA = np.where(wa <= 1, rel0[:, k:k + w],
                      np.where(wa == 3, Z_OFF,
                               np.where(wa == 4, NEG_OFF, Z_OFF)))
        iB = np.where(wa == 2, rel1[:, k:k + w], Z_OFF)
        wy = wh[:, k + w:]
        iY = np.where((wy == 1) | (wy == 2), rel1[:, k + w:],
                      np.where(wy == 3, Z_OFF, NEG_OFF))
        for arr in (iX, iA, iY, iB):
            # linearize: for col j, node i -> slot j*128+i
            chunks.append(arr.T.reshape(-1))
    lin = np.concatenate(chunks).astype(np.int16)
    assert lin.min() >= 0
    w16 = lin.reshape(-1, 16).T            # [16, TOT/16]
    return w16.copy()                      # [16, TOT/16]


def _idx_offsets(pp):
    """Column offsets (in idx units /16) of each call, per tile."""
    offs = []
    cur = 0
    for t in range(NTILES):
        d, k, w = int(pp["D"][t]), int(pp["K"][t]), int(pp["Wd"][t])
        o = {}
        for name, cols in (("X", k), ("A", w), ("Y", d - k - w), ("B", w)):
            o[name] = (cur // 16, cols * 128)
            cur += cols * 128
        offs.append(o)
    return offs, cur


# ----------------------------------------------------------------------
# Bass program
# ----------------------------------------------------------------------

def _build_program(pp, tot_idx, n_layers=4, debug_dump=False, gather_only=False, glevel=99):
    import concourse.bacc as bacc
    import concourse.bass as bass
    import concourse.mybir as mybir
    import concourse.tile as tile
    from concourse.tile import add_dep_helper
    from concourse.library_config import mlp

    dt = mybir.dt
    AF = mybir.ActivationFunctionType
    ALU = mybir.AluOpType

    D, K, Wd = pp["D"], pp["K"], pp["Wd"]
    Dmax = int(D.max())
    Wmax = max(4, int(Wd.max()))
    offs, _ = _idx_offsets(pp)

    nc = bacc.Bacc("TRN2", target_bir_lowering=False, debug=False,
                   num_devices=NC)

    t_x = nc.dram_tensor("x", [NPAD, F_IN], dt.bfloat16, kind="ExternalInput")
    t_idx = nc.dram_tensor("idx", [16, tot_idx // 16], dt.int16,
                           kind="ExternalInput")
    t_w = [nc.dram_tensor(f"W{l}", [F_IN if l == 0 else C_HID,
                                    OUT_C if l == 3 else C_HID],
                          dt.float32, kind="ExternalInput") for l in range(4)]
    t_as = [nc.dram_tensor(f"AS{l}", [128, OUT_C if l == 3 else C_HID],
                           dt.float32, kind="ExternalInput") for l in range(4)]
    t_ad = [nc.dram_tensor(f"AD{l}", [128, OUT_C if l == 3 else C_HID],
                           dt.float32, kind="ExternalInput") for l in range(4)]
    t_b = [nc.dram_tensor(f"B{l}", [128, OUT_C if l == 3 else C_HID],
                          dt.float32, kind="ExternalInput") for l in range(4)]
    t_identb = nc.dram_tensor("identb", [128, 128], dt.bfloat16,
                              kind="ExternalInput")
    t_identf = nc.dram_tensor("identf", [128, 128], dt.float32,
                              kind="ExternalInput")
    t_iotag = nc.dram_tensor("iotag", [128, 128], dt.float32,
                             kind="ExternalInput")
    t_brel = nc.dram_tensor("brel", [128, NTILES], dt.float32,
                            kind="ExternalInput")
    t_zneg = nc.dram_tensor("zneg", [2, 128], dt.bfloat16,
                            kind="ExternalInput")
    t_invc = nc.dram_tensor("invc", [128, 1], dt.float32,
                            kind="ExternalInput")
    t_wl = nc.dram_tensor("Wl", [OUT_C, 2], dt.float32, kind="ExternalInput")
    t_bl = nc.dram_tensor("bl", [128, 2], dt.float32, kind="ExternalInput")
    t_out = nc.dram_tensor("out", [G_GRAPHS, 2], dt.float32,
                           kind="ExternalOutput")
    if debug_dump:
        t_dbg_hg = nc.dram_tensor("dbg_hg", [128, 64, 128], dt.bfloat16,
                                  kind="ExternalOutput")
        t_dbg_e1 = nc.dram_tensor("dbg_e1", [128, 64 * 8], dt.float32,
                                  kind="ExternalOutput")
        t_dbg_m = nc.dram_tensor("dbg_m", [128, 64 * 72], dt.bfloat16,
                                 kind="ExternalOutput")
        t_dbg_sf = nc.dram_tensor("dbg_sf", [128, 72], dt.float32,
                                  kind="ExternalOutput")
        t_dbg_G = nc.dram_tensor("dbg_G", [16, 128], dt.bfloat16,
                                 kind="ExternalOutput")

    with tile.TileContext(nc) as tc:
        with tc.tile_pool(name="res", bufs=1) as res, \
             tc.tile_pool(name="work", bufs=2) as work, \
             tc.tile_pool(name="wk3", bufs=3) as wk3, \
             tc.tile_pool(name="ps", bufs=2, space="PSUM") as ps, \
             tc.tile_pool(name="pspool", bufs=1, space="PSUM") as pspool, \
             tc.tile_pool(name="dram", bufs=1, space="DRAM") as dram:

            nc.gpsimd.load_library(mlp)

            # ---- resident loads ----
            idx_sb = res.tile([128, tot_idx // 16], dt.int16)
            # replicate [16, T16] into the 8 partition groups on-device
            # (saves ~3.6 MB/core of upload; HW-validated pattern)
            for _g in range(8):
                nc.sync.dma_start(idx_sb[16 * _g:16 * (_g + 1), :],
                                  t_idx.ap())
            identb = res.tile([128, 128], dt.bfloat16)
            nc.sync.dma_start(identb[:], t_identb.ap())
            identf = res.tile([128, 128], dt.float32)
            nc.sync.dma_start(identf[:], t_identf.ap())
            iotag = res.tile([128, 128], dt.float32)
            nc.sync.dma_start(iotag[:], t_iotag.ap())
            brel = res.tile([128, NTILES], dt.float32)
            nc.sync.dma_start(brel[:], t_brel.ap())
            zneg = res.tile([2, 128], dt.bfloat16)
            nc.sync.dma_start(zneg[:], t_zneg.ap())
            invc = res.tile([128, 1], dt.float32)
            nc.sync.dma_start(invc[:], t_invc.ap())
            wl_sb = res.tile([OUT_C, 2], dt.float32)
            nc.sync.dma_start(wl_sb[:], t_wl.ap())
            bl_sb = res.tile([128, 2], dt.float32)
            nc.sync.dma_start(bl_sb[:], t_bl.ap())
            w_sb, as_sb, ad_sb, b_sb = [], [], [], []
            for l in range(n_layers):
                Fl = F_IN if l == 0 else C_HID
                Cl = OUT_C if l == 3 else C_HID
                w_sb.append(res.tile([Fl, Cl], dt.float32, tag=f"w{l}", name=f"w{l}"))
                nc.sync.dma_start(w_sb[l][:], t_w[l].ap())
                as_sb.append(res.tile([128, Cl], dt.float32, tag=f"as{l}", name=f"as{l}"))
                nc.sync.dma_start(as_sb[l][:], t_as[l].ap())
                ad_sb.append(res.tile([128, Cl], dt.float32, tag=f"ad{l}", name=f"ad{l}"))
                nc.sync.dma_start(ad_sb[l][:], t_ad[l].ap())
                b_sb.append(res.tile([128, Cl], dt.float32, tag=f"b{l}", name=f"bb{l}"))
                nc.sync.dma_start(b_sb[l][:], t_b[l].ap())

            f_sb = res.tile([128, NTILES, C_HID], dt.float32)
            staging = dram.tile([SHARD, 128], dt.bfloat16)

            pool_in = dram.tile([G_GRAPHS, OUT_C], dt.float32)
            pool_out = dram.tile([G_GRAPHS, OUT_C], dt.float32,
                                 addr_space="Shared")

            ps_pool = pspool.tile([128, OUT_C], dt.float32)

            prev_ag = None
            prev_gathers = []
            for l in range(n_layers):
                Fl = F_IN if l == 0 else C_HID
                Cl = OUT_C if l == 3 else C_HID
                Hl = 1 if l == 3 else HEADS
                hidl = OUT_C if l == 3 else HID
                Wm = Cl + Hl                       # m row width
                AOFF = Cl // 2                     # f32 offset of asrc field

                adst_l = res.tile([128, NTILES, HEADS], dt.float32,
                                  tag="adstl")
                pself_l = res.tile([128, NTILES, HEADS], dt.float32,
                                   tag="pselfl")
                mself_l = res.tile([128, NTILES, C_HID], dt.bfloat16,
                                   tag="mselfl")
                G = dram.tile([NC * SHARD, 128], dt.bfloat16,
                              addr_space="Shared", tag=f"G{l}", name=f"G{l}")
                stg_dmas = []

                # ---------- dense phase ----------
                for t in range(NTILES):
                    if l == 0:
                        ftb = work.tile([128, Fl], dt.bfloat16,
                                        tag="ftb")
                        nc.sync.dma_start(
                            ftb[:], t_x.ap()[t * 128:(t + 1) * 128, :])
                        ft = work.tile([128, Fl], dt.float32, tag="ft")
                        nc.vector.tensor_copy(ft[:], ftb[:])
                        fin = ft[:]
                    else:
                        fin = f_sb[:, t, :]
                    psT = ps.tile([Fl, 128], dt.float32, tag="psT")
                    nc.tensor.transpose(psT[:], fin, identf[:])
                    fT = work.tile([Fl, 128], dt.float32, tag="fT")
                    nc.scalar.activation(fT[:], psT[:], AF.Copy)
                    psH = ps.tile([128, Cl], dt.float32, tag="psH")
                    nc.tensor.matmul(psH[:], fT[:], w_sb[l][:],
                                     start=True, stop=True)
                    hw = work.tile([128, Cl], dt.float32, tag="hw")
                    nc.vector.tensor_copy(hw[:], psH[:])

                    tmp = work.tile([128, Cl], dt.float32, tag="tmp")
                    nc.vector.tensor_tensor(tmp[:], hw[:], as_sb[l][:],
                                            ALU.mult)
                    asrc_t = work.tile([128, Hl], dt.float32, tag="asrc_t")
                    nc.vector.tensor_reduce(
                        asrc_t[:],
                        tmp[:].rearrange("p (h c) -> p h c", h=Hl),
                        mybir.AxisListType.X, ALU.add)
                    nc.vector.tensor_tensor(tmp[:], hw[:], ad_sb[l][:],
                                            ALU.mult)
                    nc.vector.tensor_reduce(
                        adst_l[:, t, 0:Hl],
                        tmp[:].rearrange("p (h c) -> p h c", h=Hl),
                        mybir.AxisListType.X, ALU.add)

                    # self-loop attention term, computed without a gather:
                    # p_self = exp(leaky(asrc+adst)), m_self = p_self*h
                    es = work.tile([128, HEADS], dt.float32, tag="es")
                    nc.vector.tensor_tensor(es[:, 0:Hl], asrc_t[:],
                                            adst_l[:, t, 0:Hl], ALU.add)
                    nc.vector.scalar_tensor_tensor(
                        es[:, 0:Hl], es[:, 0:Hl], 0.2, es[:, 0:Hl],
                        ALU.mult, ALU.max)
                    nc.scalar.activation(pself_l[:, t, 0:Hl], es[:, 0:Hl],
                                         AF.Exp)
                    nc.vector.tensor_tensor(
                        mself_l[:, t, 0:Cl].rearrange("p (h c) -> p h c",
                                                      h=Hl),
                        hw[:].rearrange("p (h c) -> p h c", h=Hl),
                        pself_l[:, t, 0:Hl].unsqueeze(-1).to_broadcast(
                            (128, Hl, hidl)),
                        ALU.mult)

                    stg = work.tile([128, 128], dt.bfloat16, tag="stg")
                    nc.vector.memset(stg[:, Cl:128], 0)
                    nc.scalar.activation(stg[:, 0:Cl], hw[:], AF.Copy)
                    stgf = stg[:].bitcast(dt.float32)
                    nc.vector.tensor_copy(stgf[:, AOFF:AOFF + Hl], asrc_t[:])
                    rows = min(128, NLOC - t * 128)
                    sdma = nc.sync.dma_start(
                        staging[t * 128:t * 128 + rows, :], stg[0:rows, :])
                    stg_dmas.append(sdma)
                zdma = nc.sync.dma_start(staging[NLOC:NLOC + 2, :], zneg[:])
                stg_dmas.append(zdma)
                tc.strict_bb_all_engine_barrier()
                ag = nc.gpsimd.collective_compute(
                    "AllGather", mybir.AluOpType.bypass,
                    replica_groups=[list(range(NC))],
                    ins=[staging.opt()], outs=[G.opt()])
                tc.strict_bb_all_engine_barrier()
                prev_gathers = []
                prev_ag = ag

                # ---------- edge phase ----------
                Gap = G[:]
                G1ap = G[:][BASE1:NC * SHARD, :]
                if debug_dump and l == 0:
                    gsnap = work.tile([16, 128], dt.bfloat16, tag="gsnap")
                    g1 = nc.sync.dma_start(gsnap[0:8, :], G[NLOC - 2:NLOC + 6, :])
                    g2 = nc.sync.dma_start(gsnap[8:16, :],
                                      G[BASE1 + NLOC - 2:BASE1 + NLOC + 6, :])
                    prev_gathers.extend([g1, g2])
                    nc.sync.dma_start(t_dbg_G.ap(), gsnap[:])
                for t in range(NTILES):
                    d, k, w = int(D[t]), int(K[t]), int(Wd[t])
                    # tail region [d:d+w] holds the B (strip) gather
                    hg = wk3.tile([128, Dmax + Wmax, 128], dt.bfloat16,
                                  tag="hg")
                    o = offs[t]
                    if k + w:
                        # X and A are stream-adjacent, same base, contiguous
                        # dest: gather them in one call
                        c0, _ = o["X"] if k else o["A"]
                        n = (k + w) * 128
                        gi = nc.gpsimd.dma_gather(
                            hg[:, 0:k + w, :], Gap, idx_sb[:, c0:c0 + n // 16],
                            n, n, 128, single_packet=False)
                        prev_gathers.append(gi)
                    if (d - k - w) + w:
                        # Y and B are stream-adjacent, same base: one call
                        # lands Y at [k+w:d] and B at [d:d+w]
                        c0, _ = o["Y"] if d - k - w else o["B"]
                        n = ((d - k - w) + w) * 128
                        gi = nc.gpsimd.dma_gather(
                            hg[:, k + w:d + w, :], G1ap,
                            idx_sb[:, c0:c0 + n // 16], n, n, 128,
                            single_packet=False)
                        prev_gathers.append(gi)
                    if w:
                        nc.vector.tensor_tensor(
                            hg[:, k:k + w, 0:Cl], hg[:, k:k + w, 0:Cl],
                            hg[:, d:d + w, 0:Cl], ALU.add)
                        hgf = hg[:].bitcast(dt.float32)
                        nc.vector.tensor_tensor(
                            hgf[:, k:k + w, AOFF:AOFF + Hl],
                            hgf[:, k:k + w, AOFF:AOFF + Hl],
                            hgf[:, d:d + w, AOFF:AOFF + Hl], ALU.add)

                    if gather_only:
                        if debug_dump and l == 0 and t == 0:
                            nc.sync.dma_start(t_dbg_hg.ap()[:, 0:d, :],
                                              hg[:, 0:d, :])
                        nc.vector.memset(f_sb[:, t, :], 0.0)
                        continue
                    hgf = hg[:].bitcast(dt.float32)
                    e1 = work.tile([128, Dmax, HEADS], dt.float32, tag="e1")
                    if glevel >= 1:
                        nc.vector.tensor_tensor(
                            e1[:, 0:d, 0:Hl], hgf[:, 0:d, AOFF:AOFF + Hl],
                            adst_l[:, t, 0:Hl].unsqueeze(1).to_broadcast(
                                (128, d, Hl)), ALU.add)
                    else:
                        nc.vector.memset(e1[:, 0:d, 0:Hl], 0.0)
                    if glevel >= 2:
                        nc.vector.scalar_tensor_tensor(
                            e1[:, 0:d, 0:Hl], e1[:, 0:d, 0:Hl], 0.2,
                            e1[:, 0:d, 0:Hl], ALU.mult, ALU.max)
                    m = wk3.tile([128, Dmax, Wm], dt.bfloat16, tag="m")
                    if glevel >= 3:
                        nc.scalar.activation(m[:, 0:d, Cl:Cl + Hl],
                                             e1[:, 0:d, 0:Hl], AF.Exp)
                    else:
                        nc.vector.memset(m[:, 0:d, Cl:Cl + Hl], 1.0)
                    if glevel >= 4:
                        nc.vector.tensor_tensor(
                            m[:, 0:d, 0:Cl].rearrange(
                                "p d (h c) -> p d h c", h=Hl),
                            m[:, 0:d, Cl:Cl + Hl].unsqueeze(-1).to_broadcast(
                                (128, d, Hl, hidl)),
                            hg[:, 0:d, 0:Cl].rearrange(
                                "p d (h c) -> p d h c", h=Hl),
                            ALU.mult)
                    else:
                        nc.vector.memset(m[:, 0:d, 0:Cl], 0.0)

                    spans = []
                    for (rg0, rg1) in ((0, k), (k, k + w), (k + w, d)):
                        j = rg0
                        while j < rg1:
                            span = min(4, rg1 - j)
                            spans.append((j, span))
                            j += span
                    spans.sort(key=lambda x: -x[1])  # largest first: zeroes
                    Qn = spans[0][1]                 # the full written range
                    psA = ps.tile([128, Qn * Wm], dt.float32, tag="psA")
                    for qi, (j, span) in enumerate(spans):
                        nc.tensor.matmul(
                            psA[:, 0:span * Wm], identb[:],
                            m[:, j:j + span, :].rearrange("p a b -> p (a b)"),
                            start=(qi == 0), stop=(qi == len(spans) - 1))

                    sfin = work.tile([128, Wm], dt.float32, tag="sfin")
                    nc.vector.tensor_reduce(
                        sfin[:],
                        psA[:].rearrange("p (q w) -> p w q", q=Qn),
                        mybir.AxisListType.X, ALU.add)
                    if debug_dump and l == 0 and t == 0:
                        nc.sync.dma_start(t_dbg_hg.ap()[:, 0:d, :],
                                          hg[:, 0:d, :])
                        nc.sync.dma_start(
                            t_dbg_e1.ap()[:, 0:d * Hl],
                            e1[:, 0:d, 0:Hl].rearrange("p a b -> p (a b)"))
                        nc.sync.dma_start(
                            t_dbg_m.ap()[:, 0:d * Wm],
                            m[:, 0:d, :].rearrange("p a b -> p (a b)"))
                        nc.sync.dma_start(t_dbg_sf.ap()[:, 0:Wm], sfin[:])

                    # add the analytically-computed self-loop contribution
                    nc.vector.tensor_tensor(sfin[:, Cl:Cl + Hl],
                                            sfin[:, Cl:Cl + Hl],
                                            pself_l[:, t, 0:Hl], ALU.add)
                    nc.vector.tensor_tensor(sfin[:, 0:Cl], sfin[:, 0:Cl],
                                            mself_l[:, t, 0:Cl], ALU.add)
                    rs_t = work.tile([128, Hl], dt.float32, tag="rs_t")
                    nc.vector.reciprocal(rs_t[:], sfin[:, Cl:Cl + Hl])
                    out_t = work.tile([128, Cl], dt.float32, tag="out_t")
                    nc.vector.tensor_tensor(
                        out_t[:].rearrange("p (h c) -> p h c", h=Hl),
                        sfin[:, 0:Cl].rearrange("p (h c) -> p h c", h=Hl),
                        rs_t[:].unsqueeze(-1).to_broadcast((128, Hl, hidl)),
                        ALU.mult)
                    nc.vector.tensor_tensor(out_t[:], out_t[:], b_sb[l][:],
                                            ALU.add)
                    if l < 3:
                        ex = work.tile([128, Cl], dt.float32, tag="ex")
                        nc.scalar.activation(ex[:], out_t[:], AF.Exp)
                        nc.vector.tensor_scalar(
                            ex[:], ex[:], 1.0, -1.0, ALU.min, ALU.add)
                        t2 = work.tile([128, Cl], dt.float32, tag="t2")
                        nc.vector.tensor_scalar(
                            t2[:], out_t[:], 0.0, None, ALU.max)
                        nc.vector.tensor_tensor(
                            f_sb[:, t, :], ex[:], t2[:], ALU.add)
                    else:
                        oh = work.tile([128, 128], dt.float32, tag="oh")
                        nc.vector.tensor_tensor(
                            oh[:], iotag[:],
                            brel[:, t:t + 1].to_broadcast((128, 128)),
                            ALU.is_equal)
                        nc.tensor.matmul(ps_pool[:], oh[:], out_t[:],
                                         start=(t == 0), stop=(t == NTILES - 1))

            # ---------- pooling + final linear ----------
            if n_layers < 4:
                ps_pool_dummy = work.tile([128, OUT_C], dt.float32,
                                          tag="pool_sb2")
                nc.vector.memset(ps_pool_dummy[:], 0.0)
                nc.tensor.matmul(ps_pool[:], identf[:], ps_pool_dummy[:],
                                 start=True, stop=True)
            pool_sb = work.tile([128, OUT_C], dt.float32, tag="pool_sb")
            nc.vector.tensor_copy(pool_sb[:], ps_pool[:])
            pidma = nc.gpsimd.dma_start(pool_in[:], pool_sb[:])
            tc.strict_bb_all_engine_barrier()
            ar = nc.gpsimd.collective_compute(
                "AllReduce", mybir.AluOpType.add,
                replica_groups=[list(range(NC))],
                ins=[pool_in.opt()], outs=[pool_out.opt()])
            tc.strict_bb_all_engine_barrier()
            psum_sb = work.tile([128, OUT_C], dt.float32, tag="psum_sb")
            podma = nc.gpsimd.dma_start(psum_sb[:], pool_out[:])
            nc.vector.tensor_scalar(psum_sb[:], psum_sb[:], invc[:], None,
                                    ALU.mult)
            psT2 = ps.tile([OUT_C, 128], dt.float32, tag="psT")
            nc.tensor.transpose(psT2[:], psum_sb[:], identf[:])
            pT = work.tile([OUT_C, 128], dt.float32, tag="pT")
            nc.vector.tensor_copy(pT[:], psT2[:])
            ps_out = ps.tile([128, 2], dt.float32, tag="psH")
            nc.tensor.matmul(ps_out[:], pT[:], wl_sb[:], start=True, stop=True)
            fin_sb = work.tile([128, 2], dt.float32, tag="fin_sb")
            nc.vector.tensor_tensor(fin_sb[:], ps_out[:], bl_sb[:], ALU.add)
            nc.sync.dma_start(t_out.ap(), fin_sb[:])

    nc.compile()
    return nc


# ----------------------------------------------------------------------
# entry point
# ----------------------------------------------------------------------

def kernel(x, edge_index, batch, W0, as0, ad0, b0, W1, as1, ad1, b1,
           W2, as2, ad2, b2, Wf, asf, adf, bf, Wl, bl):
    from concourse import bass_utils

    pp = _preprocess(edge_index, batch)
    idx_cores = [_build_idx_core(pp, c) for c in range(NC)]
    _, tot_idx = _idx_offsets(pp)
    assert idx_cores[0].shape[1] * 16 == tot_idx

    nc = _build_program(pp, tot_idx)

    x = np.asarray(x, np.float32)
    weights = dict(
        W0=np.asarray(W0, np.float32), W1=np.asarray(W1, np.float32),
        W2=np.asarray(W2, np.float32), W3=np.asarray(Wf, np.float32))
    a_s = [np.asarray(a, np.float32) for a in (as0, as1, as2, asf)]
    a_d = [np.asarray(a, np.float32) for a in (ad0, ad1, ad2, adf)]
    b_l = [np.asarray(a, np.float32) for a in (b0, b1, b2, bf)]

    ident_b = np.eye(128, dtype=BF16)
    ident_f = np.eye(128, dtype=np.float32)
    iotag = np.tile(np.arange(128, dtype=np.float32)[None, :], (128, 1))
    zneg = np.zeros((2, 128), BF16)
    # NEG row: asrc field = NEG_VAL at every per-layer offset (32 and 16)
    zv = zneg.view(np.float32)
    zv[1, 32:40] = NEG_VAL   # layers 0-2: asrc field, all 8 heads
    zv[1, 16:17] = NEG_VAL   # final layer: asf field (1 head)
    invc = (1.0 / np.maximum(pp["cnt"], 1.0)).reshape(G_GRAPHS, 1)
    bl_rep = np.tile(np.asarray(bl, np.float32)[None, :], (G_GRAPHS, 1))

    in_maps = []
    for c in range(NC):
        xp = np.zeros((NPAD, F_IN), BF16)
        xp[:NLOC] = x[pp["perm"][c]].astype(BF16)
        im = dict(
            x=xp, idx=idx_cores[c],
            identb=ident_b, identf=ident_f, iotag=iotag,
            brel=pp["batchrel"][c].reshape(NTILES, 128).T.astype(np.float32),
            zneg=zneg, invc=invc.astype(np.float32),
            Wl=np.asarray(Wl, np.float32), bl=bl_rep)
        for l in range(4):
            Cl = OUT_C if l == 3 else C_HID
            im[f"W{l}"] = weights[f"W{l}"]
            im[f"AS{l}"] = np.tile(a_s[l].reshape(1, Cl), (128, 1))
            im[f"AD{l}"] = np.tile(a_d[l].reshape(1, Cl), (128, 1))
            im[f"B{l}"] = np.tile(b_l[l].reshape(1, Cl), (128, 1))
        in_maps.append(im)

    res = bass_utils.run_bass_kernel_spmd(nc, in_maps,
                                          core_ids=list(range(NC)))
    kernel.last_results = res
    kernel.last_nc = nc
    kernel.last_in_maps = in_maps
    return res.results[0]["out"]

